# revision 20
# baseline (speedup 1.0000x reference)
"""Cross-attention (B=2, N=M=2048, DIM=1024, H=16) on 8 TRN2 NeuronCores.

Sharding: tensor-parallel over heads. Core i owns heads 2i,2i+1 (128 of the
1024 q/k/v dims). Each core computes its heads' attention over all tokens,
then an on-device AllToAll redistributes head-outputs so core i holds all
1024 dims for tokens [512i, 512(i+1)); each core then applies the full
output projection for its token slice. Host unshard is a pure concat.

Compute dtype: bf16 matmul operands, f32 PSUM accumulation.
"""

import sys

for _p in ("/opt/trn_rl_repo",):
    if _p not in sys.path:
        sys.path.append(_p)

import ml_dtypes
import numpy as np

import concourse.bass as bass
import concourse.mybir as mybir
import concourse.tile as tile
from concourse import bacc

NCORES = 8
B, N, M, DIM, H = 2, 2048, 2048, 1024, 16
D = DIM // H                  # 64 head dim
HPC = H // NCORES             # 2 heads per core
DLOC = HPC * D                # 128 local q/k/v dims per core
TOK = B * N                   # 4096 query tokens (flattened b-major)
MTOK = B * M                  # 4096 key tokens
TSL = TOK // NCORES           # 512-token output slice per core
SCALE = float(D) ** -0.5

KT = DIM // 128               # 8 contraction tiles for projections
NB = 512                      # matmul moving free dim / psum bank width
SW = 1024                     # scores psum width (2 banks)
MT = M // 128                 # 16 m-tiles per batch
NCH = TOK // NB               # 8 token chunks of 512 (all-to-all granularity)

BF16 = mybir.dt.bfloat16
F32 = mybir.dt.float32
AF = mybir.ActivationFunctionType


def build(dbg=False):
    nc = bacc.Bacc("TRN2", target_bir_lowering=False, debug=False,
                   num_devices=NCORES)

    # All big inputs are host-pre-tiled so every DMA source is contiguous.
    x1t = nc.declare_dram_parameter("x1t", [KT, TOK // NB, 128, NB], BF16,
                                    isOutput=False)
    x2t = nc.declare_dram_parameter("x2t", [KT, MTOK // NB, 128, NB], BF16,
                                    isOutput=False)
    wq = nc.declare_dram_parameter("wq", [KT, 128, DLOC], BF16, isOutput=False)
    wk = nc.declare_dram_parameter("wk", [KT, 128, DLOC], BF16, isOutput=False)
    wv = nc.declare_dram_parameter("wv", [KT, 128, DLOC], BF16, isOutput=False)
    wp = nc.declare_dram_parameter("wp", [KT, 128, DIM], BF16, isOutput=False)
    bp = nc.declare_dram_parameter("bp", [1, DIM], BF16, isOutput=False)
    out = nc.declare_dram_parameter("out", [TSL, DIM], F32, isOutput=True)

    # DRAM bounce buffers for the AllToAll (collectives can't touch I/O).
    ata_in = nc.dram_tensor("ata_in", [NCORES, DLOC, TSL], BF16)
    ata_out = nc.dram_tensor("ata_out", [NCORES, DLOC, TSL], BF16)

    dbg_t = {}
    if dbg:
        dbg_t["qt"] = nc.declare_dram_parameter("dbg_qt", [128, TOK], BF16, isOutput=True)
        dbg_t["kt"] = nc.declare_dram_parameter("dbg_kt", [128, MTOK], BF16, isOutput=True)
        dbg_t["v"] = nc.declare_dram_parameter("dbg_v", [128, MTOK // 128, HPC, D + 1], BF16, isOutput=True)
        dbg_t["ot"] = nc.declare_dram_parameter("dbg_ot", [128, NCH, NB], BF16, isOutput=True)
        dbg_t["of"] = nc.declare_dram_parameter("dbg_of", [NCORES, DLOC, TSL], BF16, isOutput=True)
        dbg_t["o0"] = nc.declare_dram_parameter("dbg_o0", [D + 1, NB], F32, isOutput=True)
        dbg_t["bc0"] = nc.declare_dram_parameter("dbg_bc0", [D, NB], F32, isOutput=True)
        dbg_t["s0"] = nc.declare_dram_parameter("dbg_s0", [128, SW], F32, isOutput=True)

    with tile.TileContext(nc) as tc:
        with (
            tc.tile_pool(name="persist", bufs=1) as pp,
            tc.tile_pool(name="xin", bufs=12) as xp,
            tc.tile_pool(name="work", bufs=3) as wkp,
            tc.tile_pool(name="norm", bufs=4) as np_,
            tc.tile_pool(name="yout", bufs=3) as yp,
        ):
            # ---- persistent SBUF tensors ----
            wq_sb = pp.tile([128, KT, DLOC], BF16, tag="wq")
            wk_sb = pp.tile([128, KT, DLOC], BF16, tag="wk")
            wv_sb = pp.tile([128, KT, DLOC], BF16, tag="wv")
            wp_sb = pp.tile([128, KT, DIM], BF16, tag="wp")
            bp_sb = pp.tile([1, DIM], BF16, tag="bp")
            ones1 = pp.tile([1, 128], BF16, tag="ones1")
            qt_sb = pp.tile([128, TOK], BF16, tag="qt")     # [2 heads x 64 d, tok]
            kt_sb = pp.tile([128, MTOK], BF16, tag="kt")
            # v with ones column: [m-part, m-chunk, head, d+1]
            v_sb = pp.tile([128, MTOK // 128, HPC, D + 1], BF16, tag="v")
            ot_sb = pp.tile([128, NCH, NB], BF16, tag="ot")  # normalized head out

            for k in range(KT):
                nc.sync.dma_start(wq_sb[:, k, :], wq[k])
                nc.sync.dma_start(wk_sb[:, k, :], wk[k])
                nc.sync.dma_start(wv_sb[:, k, :], wv[k])
                nc.sync.dma_start(wp_sb[:, k, :], wp[k])
            nc.sync.dma_start(bp_sb[:], bp[:])
            nc.vector.memset(ones1[:], 1.0)
            nc.vector.memset(v_sb[:, :, :, D], 1.0)

            # ---- phase B: projections qT, kT, v ----
            with tc.tile_pool(name="ps_b", bufs=2, space="PSUM") as psb:
                for nb in range(TOK // NB):
                    sl = slice(NB * nb, NB * (nb + 1))
                    # qT [128 dloc, 512 tok] = sum_k wq_k.T @ x1t_k
                    x1_tiles = []
                    for k in range(KT):
                        xt = xp.tile([128, NB], BF16, tag="x1")
                        nc.sync.dma_start(xt[:], x1t[k, nb])
                        x1_tiles.append(xt)
                    q_ps = psb.tile([128, NB], F32, tag="qps")
                    for k in range(KT):
                        nc.tensor.matmul(q_ps[:], wq_sb[:, k, :], x1_tiles[k][:],
                                         start=(k == 0), stop=(k == KT - 1))
                    nc.any.tensor_copy(qt_sb[:, sl], q_ps[:])

                    # kT same, from x2t; keep x2 tiles for v
                    x2_tiles = []
                    for k in range(KT):
                        xt = xp.tile([128, NB], BF16, tag="x2")
                        nc.sync.dma_start(xt[:], x2t[k, nb])
                        x2_tiles.append(xt)
                    k_ps = psb.tile([128, NB], F32, tag="kps")
                    for k in range(KT):
                        nc.tensor.matmul(k_ps[:], wk_sb[:, k, :], x2_tiles[k][:],
                                         start=(k == 0), stop=(k == KT - 1))
                    nc.any.tensor_copy(kt_sb[:, sl], k_ps[:])

                    # v [m, dloc] per 128-chunk: lhsT = x2t chunk, rhs = wv
                    for j in range(NB // 128):
                        mc = (NB * nb) // 128 + j  # global m-chunk index
                        v_ps = psb.tile([128, DLOC], F32, tag="vps")
                        for k in range(KT):
                            nc.tensor.matmul(
                                v_ps[:],
                                x2_tiles[k][:, 128 * j:128 * (j + 1)],
                                wv_sb[:, k, :],
                                start=(k == 0), stop=(k == KT - 1))
                        for hh in range(HPC):
                            nc.any.tensor_copy(
                                v_sb[:, mc, hh, 0:D],
                                v_ps[:, D * hh:D * (hh + 1)])

            # ---- phase C: attention, both heads interleaved ----
            # Per (batch, 512-query-block): s_ps holds h0 scores in cols
            # [0,512) and h1 in [512,1024) — the two K=64 score matmuls sit
            # in disjoint PE row groups (base partitions 0/64) and run
            # concurrently. One wide exp covers both heads.
            with (
                tc.tile_pool(name="ps_s", bufs=2, space="PSUM") as pss,
                tc.tile_pool(name="ps_o", bufs=4, space="PSUM") as pso,
            ):
                for b in range(B):
                    for qb in range(N // NB):
                        ch = (N * b) // NB + qb  # global 512-token chunk
                        nsl = slice(N * b + NB * qb, N * b + NB * (qb + 1))
                        o_ps = [pso.tile([D + 1, NB], F32, tag="ops",
                                         name="o_ps")
                                for _ in range(HPC)]
                        for mt in range(MT):
                            msl = slice(M * b + 128 * mt,
                                        M * b + 128 * (mt + 1))
                            s_ps = pss.tile([128, HPC * NB], F32, tag="sps")
                            for hh in range(HPC):
                                hsl = slice(D * hh, D * (hh + 1))
                                nc.tensor.matmul(
                                    s_ps[:, NB * hh:NB * (hh + 1)],
                                    kt_sb[hsl, msl],
                                    qt_sb[hsl, nsl],
                                    start=True, stop=True)
                            pt = wkp.tile([128, HPC * NB], BF16, tag="pt")
                            if dbg and b == 0 and qb == 0 and mt == 0:
                                s_stage = wkp.tile([128, HPC * NB], F32,
                                                   tag="s_stage")
                                nc.any.tensor_copy(s_stage[:], s_ps[:])
                                nc.sync.dma_start(dbg_t["s0"][:], s_stage[:])
                            nc.scalar.activation(pt[:], s_ps[:], AF.Exp,
                                                 scale=SCALE)
                            for hh in range(HPC):
                                nc.tensor.matmul(
                                    o_ps[hh][:],
                                    v_sb[:, (M // 128) * b + mt, hh, :],
                                    pt[:, NB * hh:NB * (hh + 1)],
                                    start=(mt == 0), stop=(mt == MT - 1))
                        for hh in range(HPC):
                            hsl = slice(D * hh, D * (hh + 1))
                            rc = np_.tile([1, NB], F32, tag="recip")
                            nc.vector.reciprocal(rc[:], o_ps[hh][D:D + 1, :])
                            bc = np_.tile([D, NB], F32, tag="bcast")
                            nc.gpsimd.partition_broadcast(bc[:], rc[0:1, :])
                            if dbg and b == 0 and qb == 0 and hh == 0:
                                o_stage = wkp.tile([D + 1, NB], F32,
                                                   tag="o_stage")
                                nc.any.tensor_copy(o_stage[:], o_ps[hh][:])
                                nc.sync.dma_start(dbg_t["o0"][:], o_stage[:])
                                nc.sync.dma_start(dbg_t["bc0"][:], bc[:])
                            nc.vector.tensor_mul(
                                ot_sb[hsl, ch, :], o_ps[hh][0:D, :], bc[:])

            # ---- phase C': all-to-all over head-dim/token-chunks ----
            nc.sync.dma_start(
                ata_in[:].rearrange("c p t -> p c t"), ot_sb[:])
            nc.gpsimd.collective_compute(
                "AllToAll", mybir.AluOpType.bypass,
                replica_groups=[list(range(NCORES))],
                ins=[ata_in.ap().opt()],
                outs=[ata_out.ap().opt()],
            )
            of_tiles = []
            for k in range(NCORES):
                of = xp.tile([128, TSL], BF16, tag="of")
                nc.sync.dma_start(of[:], ata_out[k])
                of_tiles.append(of)

            if dbg:
                nc.sync.dma_start(dbg_t["qt"][:], qt_sb[:])
                nc.sync.dma_start(dbg_t["kt"][:], kt_sb[:])
                nc.sync.dma_start(dbg_t["v"][:], v_sb[:])
                nc.sync.dma_start(dbg_t["ot"][:], ot_sb[:])
                for k in range(NCORES):
                    nc.sync.dma_start(dbg_t["of"][k], of_tiles[k][:])

            # ---- phase D: output projection for my 512-token slice ----
            with tc.tile_pool(name="ps_y", bufs=2, space="PSUM") as psy:
                for tt in range(TSL // 128):
                    tsl_ = slice(128 * tt, 128 * (tt + 1))
                    for eb in range(DIM // NB):
                        esl = slice(NB * eb, NB * (eb + 1))
                        y_ps = psy.tile([128, NB], F32, tag="yps")
                        for k in range(NCORES):
                            nc.tensor.matmul(y_ps[:], of_tiles[k][:, tsl_],
                                             wp_sb[:, k, esl],
                                             start=(k == 0), stop=False)
                        nc.tensor.matmul(y_ps[:], ones1[:], bp_sb[:, esl],
                                         start=False, stop=True)
                        y_sb = yp.tile([128, NB], F32, tag="ysb")
                        nc.any.tensor_copy(y_sb[:], y_ps[:])
                        nc.sync.dma_start(out[tsl_, esl], y_sb[:])

    nc.compile()
    return nc


def _tile_xt(x):
    """[B,N,DIM] f32 -> [KT, TOK//NB, 128, NB] bf16 tile-contiguous x^T."""
    bf = ml_dtypes.bfloat16
    xt = x.reshape(TOK, DIM).T  # [DIM, TOK]
    return np.ascontiguousarray(
        xt.reshape(KT, 128, TOK // NB, NB).transpose(0, 2, 1, 3)).astype(bf)


def make_in_maps(x1, x2, Wq, Wkv, Wproj, bproj):
    bf = ml_dtypes.bfloat16
    x1t = _tile_xt(x1)
    x2t = _tile_xt(x2)
    wk_full = Wkv[:, :DIM]
    wv_full = Wkv[:, DIM:]
    wp = np.ascontiguousarray(Wproj.reshape(KT, 128, DIM)).astype(bf)
    bp = bproj.reshape(1, DIM).astype(bf)
    in_maps = []
    for c in range(NCORES):
        sl = slice(DLOC * c, DLOC * (c + 1))
        in_maps.append({
            "x1t": x1t, "x2t": x2t,
            "wq": np.ascontiguousarray(Wq[:, sl]).reshape(KT, 128, DLOC).astype(bf),
            "wk": np.ascontiguousarray(wk_full[:, sl]).reshape(KT, 128, DLOC).astype(bf),
            "wv": np.ascontiguousarray(wv_full[:, sl]).reshape(KT, 128, DLOC).astype(bf),
            "wp": wp, "bp": bp,
        })
    return in_maps


_nc = None


def run(inputs, trace=False, dbg=False):
    """Returns (full_output [B,N,DIM] f32, BassKernelResults)."""
    global _nc
    from concourse.bass_utils import run_bass_kernel_spmd
    if _nc is None or dbg:
        _nc = build(dbg=dbg)
    in_maps = make_in_maps(**inputs)
    res = run_bass_kernel_spmd(_nc, in_maps, core_ids=list(range(NCORES)),
                               trace=trace)
    y = np.concatenate([res.results[c]["out"] for c in range(NCORES)], axis=0)
    return y.reshape(B, N, DIM), res


def kernel(x1, x2, Wq, Wkv, Wproj, bproj):
    y, _ = run(dict(x1=x1, x2=x2, Wq=Wq, Wkv=Wkv, Wproj=Wproj, bproj=bproj))
    return y


# revision 27
# speedup vs baseline: 1.1735x; 1.1735x over previous
"""Cross-attention (B=2, N=M=2048, DIM=1024, H=16) on 8 TRN2 NeuronCores.

Sharding: tensor-parallel over heads. Core i owns heads 2i,2i+1 (128 of the
1024 q/k/v dims). Each core computes its heads' attention over all tokens,
then an on-device AllToAll redistributes head-outputs so core i holds all
1024 dims for tokens [512i, 512(i+1)); each core then applies the full
output projection for its token slice. Host unshard is a pure concat.

Compute dtype: bf16 matmul operands, f32 PSUM accumulation.
"""

import sys

for _p in ("/opt/trn_rl_repo",):
    if _p not in sys.path:
        sys.path.append(_p)

import ml_dtypes
import numpy as np

import concourse.bass as bass
import concourse.mybir as mybir
import concourse.tile as tile
from concourse import bacc

NCORES = 8
B, N, M, DIM, H = 2, 2048, 2048, 1024, 16
D = DIM // H                  # 64 head dim
HPC = H // NCORES             # 2 heads per core
DLOC = HPC * D                # 128 local q/k/v dims per core
TOK = B * N                   # 4096 query tokens (flattened b-major)
MTOK = B * M                  # 4096 key tokens
TSL = TOK // NCORES           # 512-token output slice per core
SCALE = float(D) ** -0.5

KT = DIM // 128               # 8 contraction tiles for projections
NB = 512                      # matmul moving free dim / psum bank width
SW = 1024                     # scores psum width (2 banks)
MT = M // 128                 # 16 m-tiles per batch
NCH = TOK // NB               # 8 token chunks of 512 (all-to-all granularity)

BF16 = mybir.dt.bfloat16
F32 = mybir.dt.float32
AF = mybir.ActivationFunctionType


def build(dbg=False):
    nc = bacc.Bacc("TRN2", target_bir_lowering=False, debug=False,
                   num_devices=NCORES)

    # All big inputs are host-pre-tiled so every DMA source is contiguous:
    # x?t[nb] is one [128, KT, NB] block — a single 1 MB DMA per 512-token
    # block (DMA issue on the Sync engine costs ~0.7 us each, so few big
    # DMAs beat many small ones).
    x1t = nc.declare_dram_parameter("x1t", [TOK // NB, 128, KT, NB], BF16,
                                    isOutput=False)
    x2t = nc.declare_dram_parameter("x2t", [MTOK // NB, 128, KT, NB], BF16,
                                    isOutput=False)
    wq = nc.declare_dram_parameter("wq", [KT, 128, DLOC], BF16, isOutput=False)
    wk = nc.declare_dram_parameter("wk", [KT, 128, DLOC], BF16, isOutput=False)
    wv = nc.declare_dram_parameter("wv", [KT, 128, DLOC], BF16, isOutput=False)
    wp = nc.declare_dram_parameter("wp", [KT, 128, DIM], BF16, isOutput=False)
    bp = nc.declare_dram_parameter("bp", [1, DIM], BF16, isOutput=False)
    out = nc.declare_dram_parameter("out", [TSL, DIM], F32, isOutput=True)

    # DRAM bounce buffers for the AllToAll (collectives can't touch I/O).
    ata_in = nc.dram_tensor("ata_in", [NCORES, DLOC, TSL], BF16)
    ata_out = nc.dram_tensor("ata_out", [NCORES, DLOC, TSL], BF16)

    dbg_t = {}
    if dbg:
        dbg_t["qt"] = nc.declare_dram_parameter("dbg_qt", [128, TOK], BF16, isOutput=True)
        dbg_t["kt"] = nc.declare_dram_parameter("dbg_kt", [128, MTOK], BF16, isOutput=True)
        dbg_t["v"] = nc.declare_dram_parameter("dbg_v", [128, MTOK // 128, HPC, D + 1], BF16, isOutput=True)
        dbg_t["ot"] = nc.declare_dram_parameter("dbg_ot", [128, NCH, NB], BF16, isOutput=True)
        dbg_t["of"] = nc.declare_dram_parameter("dbg_of", [NCORES, DLOC, TSL], BF16, isOutput=True)
        dbg_t["o0"] = nc.declare_dram_parameter("dbg_o0", [D + 1, NB], F32, isOutput=True)
        dbg_t["bc0"] = nc.declare_dram_parameter("dbg_bc0", [D, NB], F32, isOutput=True)
        dbg_t["s0"] = nc.declare_dram_parameter("dbg_s0", [128, SW], F32, isOutput=True)

    with tile.TileContext(nc) as tc:
        with (
            tc.tile_pool(name="persist", bufs=1) as pp,
            tc.tile_pool(name="xin", bufs=12) as xp,
            tc.tile_pool(name="work", bufs=3) as wkp,
            tc.tile_pool(name="norm", bufs=4) as np_,
            tc.tile_pool(name="yout", bufs=3) as yp,
        ):
            # ---- persistent SBUF tensors ----
            wq_sb = pp.tile([128, KT, DLOC], BF16, tag="wq")
            wk_sb = pp.tile([128, KT, DLOC], BF16, tag="wk")
            wv_sb = pp.tile([128, KT, DLOC], BF16, tag="wv")
            wp_sb = pp.tile([128, KT, DIM], BF16, tag="wp")
            bp_sb = pp.tile([1, DIM], BF16, tag="bp")
            ones1 = pp.tile([1, 128], BF16, tag="ones1")
            qt_sb = pp.tile([128, TOK], BF16, tag="qt")     # [2 heads x 64 d, tok]
            kt_sb = pp.tile([128, MTOK], BF16, tag="kt")
            # v with ones column: [m-part, m-chunk, head, d+1]
            v_sb = pp.tile([128, MTOK // 128, HPC, D + 1], BF16, tag="v")
            ot_sb = pp.tile([128, NCH, NB], BF16, tag="ot")  # normalized head out

            for k in range(KT):
                nc.sync.dma_start(wq_sb[:, k, :], wq[k])
                nc.sync.dma_start(wk_sb[:, k, :], wk[k])
                nc.sync.dma_start(wv_sb[:, k, :], wv[k])
                nc.sync.dma_start(wp_sb[:, k, :], wp[k])
            nc.sync.dma_start(bp_sb[:], bp[:])
            nc.vector.memset(ones1[:], 1.0)
            nc.vector.memset(v_sb[:, :, :, D], 1.0)

            # ---- phase B: projections qT, kT, v ----
            with tc.tile_pool(name="ps_b", bufs=2, space="PSUM") as psb:
                for nb in range(TOK // NB):
                    sl = slice(NB * nb, NB * (nb + 1))
                    # qT [128 dloc, 512 tok] = sum_k wq_k.T @ x1t_k
                    x1_t = xp.tile([128, KT, NB], BF16, tag="x1", bufs=3)
                    nc.sync.dma_start(x1_t[:], x1t[nb])
                    q_ps = psb.tile([128, NB], F32, tag="qps")
                    for k in range(KT):
                        nc.tensor.matmul(q_ps[:], wq_sb[:, k, :], x1_t[:, k, :],
                                         start=(k == 0), stop=(k == KT - 1))
                    nc.vector.tensor_copy(qt_sb[:, sl], q_ps[:])

                    # kT same, from x2t; keep x2 tiles for v
                    x2_t = xp.tile([128, KT, NB], BF16, tag="x2", bufs=3)
                    nc.sync.dma_start(x2_t[:], x2t[nb])
                    k_ps = psb.tile([128, NB], F32, tag="kps")
                    for k in range(KT):
                        nc.tensor.matmul(k_ps[:], wk_sb[:, k, :], x2_t[:, k, :],
                                         start=(k == 0), stop=(k == KT - 1))
                    nc.vector.tensor_copy(kt_sb[:, sl], k_ps[:])

                    # v [m, dloc] per 128-chunk: lhsT = x2t chunk, rhs = wv
                    for j in range(NB // 128):
                        mc = (NB * nb) // 128 + j  # global m-chunk index
                        v_ps = psb.tile([128, DLOC], F32, tag="vps")
                        for k in range(KT):
                            nc.tensor.matmul(
                                v_ps[:],
                                x2_t[:, k, 128 * j:128 * (j + 1)],
                                wv_sb[:, k, :],
                                start=(k == 0), stop=(k == KT - 1))
                        for hh in range(HPC):
                            nc.vector.tensor_copy(
                                v_sb[:, mc, hh, 0:D],
                                v_ps[:, D * hh:D * (hh + 1)])

            # ---- phase C: attention, both heads interleaved ----
            # Per (batch, 512-query-block): s_ps holds h0 scores in cols
            # [0,512) and h1 in [512,1024) — the two K=64 score matmuls sit
            # in disjoint PE row groups (base partitions 0/64) and run
            # concurrently. One wide exp covers both heads.
            with (
                tc.tile_pool(name="ps_s", bufs=2, space="PSUM") as pss,
                tc.tile_pool(name="ps_o", bufs=4, space="PSUM") as pso,
            ):
                for b in range(B):
                    for qb in range(N // NB):
                        ch = (N * b) // NB + qb  # global 512-token chunk
                        nsl = slice(N * b + NB * qb, N * b + NB * (qb + 1))
                        o_ps = [pso.tile([D + 1, NB], F32, tag="ops",
                                         name="o_ps")
                                for _ in range(HPC)]
                        for mt in range(MT):
                            msl = slice(M * b + 128 * mt,
                                        M * b + 128 * (mt + 1))
                            s_ps = pss.tile([128, HPC * NB], F32, tag="sps")
                            for hh in range(HPC):
                                hsl = slice(D * hh, D * (hh + 1))
                                nc.tensor.matmul(
                                    s_ps[:, NB * hh:NB * (hh + 1)],
                                    kt_sb[hsl, msl],
                                    qt_sb[hsl, nsl],
                                    start=True, stop=True)
                            pt = wkp.tile([128, HPC * NB], BF16, tag="pt")
                            if dbg and b == 0 and qb == 0 and mt == 0:
                                s_stage = wkp.tile([128, HPC * NB], F32,
                                                   tag="s_stage")
                                nc.any.tensor_copy(s_stage[:], s_ps[:])
                                nc.sync.dma_start(dbg_t["s0"][:], s_stage[:])
                            nc.scalar.activation(pt[:], s_ps[:], AF.Exp,
                                                 scale=SCALE)
                            for hh in range(HPC):
                                nc.tensor.matmul(
                                    o_ps[hh][:],
                                    v_sb[:, (M // 128) * b + mt, hh, :],
                                    pt[:, NB * hh:NB * (hh + 1)],
                                    start=(mt == 0), stop=(mt == MT - 1))
                        for hh in range(HPC):
                            hsl = slice(D * hh, D * (hh + 1))
                            rc = np_.tile([1, NB], F32, tag="recip")
                            nc.vector.reciprocal(rc[:], o_ps[hh][D:D + 1, :])
                            bc = np_.tile([D, NB], F32, tag="bcast")
                            nc.gpsimd.partition_broadcast(bc[:], rc[0:1, :])
                            if dbg and b == 0 and qb == 0 and hh == 0:
                                o_stage = wkp.tile([D + 1, NB], F32,
                                                   tag="o_stage")
                                nc.any.tensor_copy(o_stage[:], o_ps[hh][:])
                                nc.sync.dma_start(dbg_t["o0"][:], o_stage[:])
                                nc.sync.dma_start(dbg_t["bc0"][:], bc[:])
                            nc.vector.tensor_mul(
                                ot_sb[hsl, ch, :], o_ps[hh][0:D, :], bc[:])

            # ---- phase C': all-to-all over head-dim/token-chunks ----
            nc.sync.dma_start(
                ata_in[:].rearrange("c p t -> p c t"), ot_sb[:])
            nc.gpsimd.collective_compute(
                "AllToAll", mybir.AluOpType.bypass,
                replica_groups=[list(range(NCORES))],
                ins=[ata_in.ap().opt()],
                outs=[ata_out.ap().opt()],
            )
            of_tiles = []
            for k in range(NCORES):
                of = xp.tile([128, TSL], BF16, tag="of", bufs=8)
                nc.sync.dma_start(of[:], ata_out[k])
                of_tiles.append(of)

            if dbg:
                nc.sync.dma_start(dbg_t["qt"][:], qt_sb[:])
                nc.sync.dma_start(dbg_t["kt"][:], kt_sb[:])
                nc.sync.dma_start(dbg_t["v"][:], v_sb[:])
                nc.sync.dma_start(dbg_t["ot"][:], ot_sb[:])
                for k in range(NCORES):
                    nc.sync.dma_start(dbg_t["of"][k], of_tiles[k][:])

            # ---- phase D: output projection for my 512-token slice ----
            with tc.tile_pool(name="ps_y", bufs=2, space="PSUM") as psy:
                for tt in range(TSL // 128):
                    tsl_ = slice(128 * tt, 128 * (tt + 1))
                    for eb in range(DIM // NB):
                        esl = slice(NB * eb, NB * (eb + 1))
                        y_ps = psy.tile([128, NB], F32, tag="yps")
                        for k in range(NCORES):
                            nc.tensor.matmul(y_ps[:], of_tiles[k][:, tsl_],
                                             wp_sb[:, k, esl],
                                             start=(k == 0), stop=False)
                        nc.tensor.matmul(y_ps[:], ones1[:], bp_sb[:, esl],
                                         start=False, stop=True)
                        y_sb = yp.tile([128, NB], F32, tag="ysb")
                        nc.vector.tensor_copy(y_sb[:], y_ps[:])
                        nc.sync.dma_start(out[tsl_, esl], y_sb[:])

    nc.compile()
    return nc


def _tile_xt(x):
    """[B,N,DIM] f32 -> [TOK//NB, 128, KT, NB] bf16 block-contiguous x^T."""
    bf = ml_dtypes.bfloat16
    xt = x.reshape(TOK, DIM).T  # [DIM, TOK]
    return np.ascontiguousarray(
        xt.reshape(KT, 128, TOK // NB, NB).transpose(2, 1, 0, 3)).astype(bf)


def make_in_maps(x1, x2, Wq, Wkv, Wproj, bproj):
    bf = ml_dtypes.bfloat16
    x1t = _tile_xt(x1)
    x2t = _tile_xt(x2)
    wk_full = Wkv[:, :DIM]
    wv_full = Wkv[:, DIM:]
    wp = np.ascontiguousarray(Wproj.reshape(KT, 128, DIM)).astype(bf)
    bp = bproj.reshape(1, DIM).astype(bf)
    in_maps = []
    for c in range(NCORES):
        sl = slice(DLOC * c, DLOC * (c + 1))
        in_maps.append({
            "x1t": x1t, "x2t": x2t,
            "wq": np.ascontiguousarray(Wq[:, sl]).reshape(KT, 128, DLOC).astype(bf),
            "wk": np.ascontiguousarray(wk_full[:, sl]).reshape(KT, 128, DLOC).astype(bf),
            "wv": np.ascontiguousarray(wv_full[:, sl]).reshape(KT, 128, DLOC).astype(bf),
            "wp": wp, "bp": bp,
        })
    return in_maps


_nc = None


def run(inputs, trace=False, dbg=False):
    """Returns (full_output [B,N,DIM] f32, BassKernelResults)."""
    global _nc
    from concourse.bass_utils import run_bass_kernel_spmd
    if _nc is None or dbg:
        _nc = build(dbg=dbg)
    in_maps = make_in_maps(**inputs)
    res = run_bass_kernel_spmd(_nc, in_maps, core_ids=list(range(NCORES)),
                               trace=trace)
    y = np.concatenate([res.results[c]["out"] for c in range(NCORES)], axis=0)
    return y.reshape(B, N, DIM), res


def kernel(x1, x2, Wq, Wkv, Wproj, bproj):
    y, _ = run(dict(x1=x1, x2=x2, Wq=Wq, Wkv=Wkv, Wproj=Wproj, bproj=bproj))
    return y


# revision 37
# speedup vs baseline: 1.1751x; 1.0013x over previous
"""Cross-attention (B=2, N=M=2048, DIM=1024, H=16) on 8 TRN2 NeuronCores.

Sharding: tensor-parallel over heads. Core i owns heads 2i,2i+1 (128 of the
1024 q/k/v dims). Each core computes its heads' attention over all tokens,
then an on-device AllToAll redistributes head-outputs so core i holds all
1024 dims for tokens [512i, 512(i+1)); each core then applies the full
output projection for its token slice. Host unshard is a pure concat.

Compute dtype: bf16 matmul operands, f32 PSUM accumulation.
"""

import sys

for _p in ("/opt/trn_rl_repo",):
    if _p not in sys.path:
        sys.path.append(_p)

import ml_dtypes
import numpy as np

import concourse.bass as bass
import concourse.mybir as mybir
import concourse.tile as tile
from concourse import bacc

NCORES = 8
B, N, M, DIM, H = 2, 2048, 2048, 1024, 16
D = DIM // H                  # 64 head dim
HPC = H // NCORES             # 2 heads per core
DLOC = HPC * D                # 128 local q/k/v dims per core
TOK = B * N                   # 4096 query tokens (flattened b-major)
MTOK = B * M                  # 4096 key tokens
TSL = TOK // NCORES           # 512-token output slice per core
SCALE = float(D) ** -0.5

KT = DIM // 128               # 8 contraction tiles for projections
NB = 512                      # matmul moving free dim / psum bank width
SW = 1024                     # scores psum width (2 banks)
MT = M // 128                 # 16 m-tiles per batch
NCH = TOK // NB               # 8 token chunks of 512 (all-to-all granularity)

BF16 = mybir.dt.bfloat16
F32 = mybir.dt.float32
AF = mybir.ActivationFunctionType


def build(dbg=False):
    nc = bacc.Bacc("TRN2", target_bir_lowering=False, debug=False,
                   num_devices=NCORES)

    # All big inputs are host-pre-tiled so every DMA source is contiguous:
    # x?t[nb] is one [128, KT, NB] block — a single 1 MB DMA per 512-token
    # block (DMA issue on the Sync engine costs ~0.7 us each, so few big
    # DMAs beat many small ones).
    x1t = nc.declare_dram_parameter("x1t", [TOK // NB, 128, KT, NB], BF16,
                                    isOutput=False)
    x2t = nc.declare_dram_parameter("x2t", [MTOK // NB, 128, KT, NB], BF16,
                                    isOutput=False)
    wq = nc.declare_dram_parameter("wq", [KT, 128, DLOC], BF16, isOutput=False)
    wk = nc.declare_dram_parameter("wk", [KT, 128, DLOC], BF16, isOutput=False)
    wv = nc.declare_dram_parameter("wv", [KT, 128, DLOC], BF16, isOutput=False)
    wp = nc.declare_dram_parameter("wp", [KT, 128, DIM], BF16, isOutput=False)
    bp = nc.declare_dram_parameter("bp", [1, DIM], BF16, isOutput=False)
    out = nc.declare_dram_parameter("out", [TSL, DIM], F32, isOutput=True)

    # DRAM bounce buffers for the AllToAll (collectives can't touch I/O).
    ata_in = nc.dram_tensor("ata_in", [NCORES, DLOC, TSL], BF16)
    ata_out = nc.dram_tensor("ata_out", [NCORES, DLOC, TSL], BF16)

    dbg_t = {}
    if dbg:
        dbg_t["qt"] = nc.declare_dram_parameter("dbg_qt", [128, TOK], BF16, isOutput=True)
        dbg_t["kt"] = nc.declare_dram_parameter("dbg_kt", [128, MTOK], BF16, isOutput=True)
        dbg_t["v"] = nc.declare_dram_parameter("dbg_v", [128, MTOK // 128, HPC, D + 1], BF16, isOutput=True)
        dbg_t["ot"] = nc.declare_dram_parameter("dbg_ot", [128, NCH, NB], BF16, isOutput=True)
        dbg_t["of"] = nc.declare_dram_parameter("dbg_of", [NCORES, DLOC, TSL], BF16, isOutput=True)
        dbg_t["o0"] = nc.declare_dram_parameter("dbg_o0", [D + 1, NB], F32, isOutput=True)
        dbg_t["bc0"] = nc.declare_dram_parameter("dbg_bc0", [D, NB], F32, isOutput=True)
        dbg_t["s0"] = nc.declare_dram_parameter("dbg_s0", [128, SW], F32, isOutput=True)

    with tile.TileContext(nc) as tc:
        with (
            tc.tile_pool(name="persist", bufs=1) as pp,
            tc.tile_pool(name="xin", bufs=12) as xp,
            tc.tile_pool(name="work", bufs=3) as wkp,
            tc.tile_pool(name="norm", bufs=4) as np_,
            tc.tile_pool(name="yout", bufs=3) as yp,
        ):
            # ---- persistent SBUF tensors ----
            wq_sb = pp.tile([128, KT, DLOC], BF16, tag="wq")
            wk_sb = pp.tile([128, KT, DLOC], BF16, tag="wk")
            wv_sb = pp.tile([128, KT, DLOC], BF16, tag="wv")
            wp_sb = pp.tile([128, KT, DIM], BF16, tag="wp")
            bp_sb = pp.tile([1, DIM], BF16, tag="bp")
            ones1 = pp.tile([1, 128], BF16, tag="ones1")
            # per-batch tensors so phase C(b) only depends on phase B(b)
            qt_b = [pp.tile([128, N], BF16, tag=f"qt{b}", name=f"qt{b}")
                    for b in range(B)]
            kt_b = [pp.tile([128, M], BF16, tag=f"kt{b}", name=f"kt{b}")
                    for b in range(B)]
            # v with ones column: [m-part, m-chunk, head, d+1]
            v_b = [pp.tile([128, M // 128, HPC, D + 1], BF16, tag=f"v{b}",
                           name=f"v{b}")
                   for b in range(B)]
            ot_sb = pp.tile([128, NCH, NB], BF16, tag="ot")  # normalized head out

            for k in range(KT):
                nc.sync.dma_start(wq_sb[:, k, :], wq[k])
                nc.sync.dma_start(wk_sb[:, k, :], wk[k])
                nc.sync.dma_start(wv_sb[:, k, :], wv[k])
                nc.sync.dma_start(wp_sb[:, k, :], wp[k])
            nc.sync.dma_start(bp_sb[:], bp[:])
            nc.vector.memset(ones1[:], 1.0)
            for b in range(B):
                nc.vector.memset(v_b[b][:, :, :, D], 1.0)

            # ---- phase B: projections qT, kT, v ----
            with tc.tile_pool(name="ps_b", bufs=2, space="PSUM") as psb:
                for nb in range(TOK // NB):
                    sl = slice(NB * nb, NB * (nb + 1))
                    # qT [128 dloc, 512 tok] = sum_k wq_k.T @ x1t_k
                    x1_t = xp.tile([128, KT, NB], BF16, tag="x1", bufs=3)
                    nc.sync.dma_start(x1_t[:], x1t[nb])
                    q_ps = psb.tile([128, NB], F32, tag="qps")
                    for k in range(KT):
                        nc.tensor.matmul(q_ps[:], wq_sb[:, k, :], x1_t[:, k, :],
                                         start=(k == 0), stop=(k == KT - 1))
                    bb, lsl = nb // (N // NB), slice(NB * (nb % (N // NB)),
                                                    NB * (nb % (N // NB)) + NB)
                    nc.vector.tensor_copy(qt_b[bb][:, lsl], q_ps[:])

                    # kT same, from x2t; keep x2 tiles for v
                    x2_t = xp.tile([128, KT, NB], BF16, tag="x2", bufs=3)
                    nc.sync.dma_start(x2_t[:], x2t[nb])
                    k_ps = psb.tile([128, NB], F32, tag="kps")
                    for k in range(KT):
                        nc.tensor.matmul(k_ps[:], wk_sb[:, k, :], x2_t[:, k, :],
                                         start=(k == 0), stop=(k == KT - 1))
                    nc.vector.tensor_copy(kt_b[bb][:, lsl], k_ps[:])

                    # v [m, dloc] per 128-chunk: lhsT = x2t chunk, rhs = wv
                    for j in range(NB // 128):
                        mc = (NB * nb) // 128 + j  # global m-chunk index
                        v_ps = psb.tile([128, DLOC], F32, tag="vps")
                        for k in range(KT):
                            nc.tensor.matmul(
                                v_ps[:],
                                x2_t[:, k, 128 * j:128 * (j + 1)],
                                wv_sb[:, k, :],
                                start=(k == 0), stop=(k == KT - 1))
                        for hh in range(HPC):
                            nc.vector.tensor_copy(
                                v_b[mc // (M // 128)][:, mc % (M // 128), hh, 0:D],
                                v_ps[:, D * hh:D * (hh + 1)])

            # ---- phase C: attention, both heads interleaved ----
            # Per (batch, 512-query-block): s_ps holds h0 scores in cols
            # [0,512) and h1 in [512,1024) — the two K=64 score matmuls sit
            # in disjoint PE row groups (base partitions 0/64) and run
            # concurrently. One wide exp covers both heads.
            with (
                tc.tile_pool(name="ps_s", bufs=2, space="PSUM") as pss,
                tc.tile_pool(name="ps_o", bufs=4, space="PSUM") as pso,
            ):
                for b in range(B):
                    for qb in range(N // NB):
                        ch = (N * b) // NB + qb  # global 512-token chunk
                        nsl = slice(N * b + NB * qb, N * b + NB * (qb + 1))
                        o_ps = [pso.tile([D + 1, NB], F32, tag="ops",
                                         name="o_ps")
                                for _ in range(HPC)]
                        lnsl = slice(NB * qb, NB * (qb + 1))
                        for mt in range(MT):
                            msl = slice(128 * mt, 128 * (mt + 1))
                            s_ps = pss.tile([128, HPC * NB], F32, tag="sps")
                            for hh in range(HPC):
                                hsl = slice(D * hh, D * (hh + 1))
                                nc.tensor.matmul(
                                    s_ps[:, NB * hh:NB * (hh + 1)],
                                    kt_b[b][hsl, msl],
                                    qt_b[b][hsl, lnsl],
                                    start=True, stop=True)
                            pt = wkp.tile([128, HPC * NB], BF16, tag="pt")
                            if dbg and b == 0 and qb == 0 and mt == 0:
                                s_stage = wkp.tile([128, HPC * NB], F32,
                                                   tag="s_stage")
                                nc.any.tensor_copy(s_stage[:], s_ps[:])
                                nc.sync.dma_start(dbg_t["s0"][:], s_stage[:])
                            nc.scalar.activation(pt[:], s_ps[:], AF.Exp,
                                                 scale=SCALE)
                            for hh in range(HPC):
                                nc.tensor.matmul(
                                    o_ps[hh][:],
                                    v_b[b][:, mt, hh, :],
                                    pt[:, NB * hh:NB * (hh + 1)],
                                    start=(mt == 0), stop=(mt == MT - 1))
                        for hh in range(HPC):
                            hsl = slice(D * hh, D * (hh + 1))
                            rc = np_.tile([1, NB], F32, tag="recip")
                            nc.vector.reciprocal(rc[:], o_ps[hh][D:D + 1, :])
                            bc = np_.tile([D, NB], F32, tag="bcast")
                            nc.gpsimd.partition_broadcast(bc[:], rc[0:1, :])
                            if dbg and b == 0 and qb == 0 and hh == 0:
                                o_stage = wkp.tile([D + 1, NB], F32,
                                                   tag="o_stage")
                                nc.any.tensor_copy(o_stage[:], o_ps[hh][:])
                                nc.sync.dma_start(dbg_t["o0"][:], o_stage[:])
                                nc.sync.dma_start(dbg_t["bc0"][:], bc[:])
                            nc.vector.tensor_mul(
                                ot_sb[hsl, ch, :], o_ps[hh][0:D, :], bc[:])

            # ---- phase C': all-to-all over head-dim/token-chunks ----
            nc.sync.dma_start(
                ata_in[:].rearrange("c p t -> p c t"), ot_sb[:])
            nc.gpsimd.collective_compute(
                "AllToAll", mybir.AluOpType.bypass,
                replica_groups=[list(range(NCORES))],
                ins=[ata_in.ap().opt()],
                outs=[ata_out.ap().opt()],
            )
            of_tiles = []
            for k in range(NCORES):
                of = xp.tile([128, TSL], BF16, tag="of", bufs=8)
                nc.sync.dma_start(of[:], ata_out[k])
                of_tiles.append(of)

            if dbg:
                for b in range(B):
                    nc.sync.dma_start(dbg_t["qt"][:, N * b:N * (b + 1)], qt_b[b][:])
                    nc.sync.dma_start(dbg_t["kt"][:, M * b:M * (b + 1)], kt_b[b][:])
                    nc.sync.dma_start(
                        dbg_t["v"][:, (M // 128) * b:(M // 128) * (b + 1)], v_b[b][:])
                nc.sync.dma_start(dbg_t["ot"][:], ot_sb[:])
                for k in range(NCORES):
                    nc.sync.dma_start(dbg_t["of"][k], of_tiles[k][:])

            # ---- phase D: output projection for my 512-token slice ----
            with tc.tile_pool(name="ps_y", bufs=2, space="PSUM") as psy:
                # keep the PE busy (HAM-warm) while the collective flies;
                # results are never read
                dum_ps = psy.tile([128, NB], F32, tag="dum")
                for _ in range(56):
                    # reads the last ot chunk so these only start at C's end
                    nc.tensor.matmul(dum_ps[:], ot_sb[:, NCH - 1, 0:128],
                                     ot_sb[:, NCH - 1, :], start=True, stop=True)
                for tt in range(TSL // 128):
                    tsl_ = slice(128 * tt, 128 * (tt + 1))
                    for eb in range(DIM // NB):
                        esl = slice(NB * eb, NB * (eb + 1))
                        y_ps = psy.tile([128, NB], F32, tag="yps")
                        nc.tensor.matmul(y_ps[:], ones1[:], bp_sb[:, esl],
                                         start=True, stop=False)
                        for k in range(NCORES):
                            nc.tensor.matmul(y_ps[:], of_tiles[k][:, tsl_],
                                             wp_sb[:, k, esl],
                                             start=False, stop=(k == NCORES - 1))
                        y_sb = yp.tile([128, NB], F32, tag="ysb")
                        nc.vector.tensor_copy(y_sb[:], y_ps[:])
                        nc.sync.dma_start(out[tsl_, esl], y_sb[:])

    nc.compile()
    return nc


def _tile_xt(x):
    """[B,N,DIM] f32 -> [TOK//NB, 128, KT, NB] bf16 block-contiguous x^T."""
    bf = ml_dtypes.bfloat16
    xt = x.reshape(TOK, DIM).T  # [DIM, TOK]
    return np.ascontiguousarray(
        xt.reshape(KT, 128, TOK // NB, NB).transpose(2, 1, 0, 3)).astype(bf)


def make_in_maps(x1, x2, Wq, Wkv, Wproj, bproj):
    bf = ml_dtypes.bfloat16
    x1t = _tile_xt(x1)
    x2t = _tile_xt(x2)
    wk_full = Wkv[:, :DIM]
    wv_full = Wkv[:, DIM:]
    wp = np.ascontiguousarray(Wproj.reshape(KT, 128, DIM)).astype(bf)
    bp = bproj.reshape(1, DIM).astype(bf)
    in_maps = []
    for c in range(NCORES):
        sl = slice(DLOC * c, DLOC * (c + 1))
        in_maps.append({
            "x1t": x1t, "x2t": x2t,
            "wq": np.ascontiguousarray(Wq[:, sl]).reshape(KT, 128, DLOC).astype(bf),
            "wk": np.ascontiguousarray(wk_full[:, sl]).reshape(KT, 128, DLOC).astype(bf),
            "wv": np.ascontiguousarray(wv_full[:, sl]).reshape(KT, 128, DLOC).astype(bf),
            "wp": wp, "bp": bp,
        })
    return in_maps


_nc = None


def run(inputs, trace=False, dbg=False):
    """Returns (full_output [B,N,DIM] f32, BassKernelResults)."""
    global _nc
    from concourse.bass_utils import run_bass_kernel_spmd
    if _nc is None or dbg:
        _nc = build(dbg=dbg)
    in_maps = make_in_maps(**inputs)
    res = run_bass_kernel_spmd(_nc, in_maps, core_ids=list(range(NCORES)),
                               trace=trace)
    y = np.concatenate([res.results[c]["out"] for c in range(NCORES)], axis=0)
    return y.reshape(B, N, DIM), res


def kernel(x1, x2, Wq, Wkv, Wproj, bproj):
    y, _ = run(dict(x1=x1, x2=x2, Wq=Wq, Wkv=Wkv, Wproj=Wproj, bproj=bproj))
    return y
